# revision 1
# baseline (speedup 1.0000x reference)
"""Trainium2 Bass kernel for nn_DCTFeatureModel.

Math: the reference pipeline (3D DCT-II over [time-in-bin, H, W], mean over
DCT bins, full-receptive-field Conv3d, bias, LeakyReLU) is linear up to the
LeakyReLU, so everything folds into a single small matmul:

    feat[b,s,o] = LeakyReLU( sum_{c,t,i,j} x[b,s,c,t,i,j] * Weff[s,o,t,i,j]
                             + bias[s,o] )
    Weff[s,o,t,i,j] = (1/8) * sum_{f,p,q} Ct[f,t] Cs[p,i] Cs[q,j] W[s,o,f,p,q]

Weff is tiny (2*64*2048 floats) and computed on host. The device kernel is
memory-bound: stream x (134 MB full / 16.8 MB per core), reduce over the 8
DCT bins (c), then a [128b x 2048k] @ [2048k x 64o] matmul per subwindow.

Device dataflow (per core, fp32 exact): the host lays each core's x shard
out as contiguous [kin=128, chin*b = 1024] blocks per (s, c, g) so every
DMA unit is a fully contiguous 512 KB load arriving with the contraction
index already on partitions, and all 8 c-blocks of a (s, g) group land
within a ~10 us window. The c-reduction is a binary tree per group split
across DVE and GPSIMD (each engine's chain kept engine-local, one cross
join at the root); each reduced tile's 128-col slices are directly the
transposed matmul operands (no PE transposes, no PSUM->SBUF copies).
PE does 32 accumulating fp32 matmuls + 2 rank-1 bias matmuls;
LeakyReLU = max(v, 0.02v) on DVE.

Sharding: pure data-parallel over batch, 1024/8 = 128 rows per core.
"""

from contextlib import ExitStack

import numpy as np

import concourse.bacc as bacc
import concourse.tile as tile
from concourse import mybir
from concourse.bass_utils import run_bass_kernel_spmd

# Problem shapes (hardcoded per contract)
B = 1024
NCORES = 8
BS = B // NCORES          # 128 batch rows per core
NSW = 2                   # subwindows
NBINS = 8                 # DCT bins (mean-reduced)
NDCT = 32                 # time points per bin
HW = 8
NF = 64                   # conv output filters per subwindow
K = NDCT * HW * HW        # 2048 contraction elements per (s, c)
P = 128                   # partitions
NCHUNK = K // P           # 16 k-chunks of 128
NG = 2                    # chunk-groups per s
CPG = NCHUNK // NG        # 8 chunks per group
GW = CPG * P              # 1024 columns per group tile
OUT_F = NSW * NF          # 128 output features
SLOPE = 0.02

F32 = mybir.dt.float32

_cached = None
last_results = None


def _dct2(N):
    n = np.arange(N, dtype=np.float64)
    k = np.arange(N, dtype=np.float64)
    return 2.0 * np.cos(np.pi * (2.0 * n[None, :] + 1.0) * k[:, None] / (2.0 * N))


def _kernel_body(tc, x, w, bias, out):
    """x: [NSW*NBINS*NG, 128, GW] (s,c,g blocks, each [kin, chin*b], contiguous)
    w: [P, NSW*NCHUNK*NF]; bias: [1, OUT_F]; out: [BS, OUT_F]"""
    nc = tc.nc
    with ExitStack() as ctx:
        const_pool = ctx.enter_context(tc.tile_pool(name="const", bufs=1))
        xpool = ctx.enter_context(tc.tile_pool(name="xp", bufs=14))
        tpool = ctx.enter_context(tc.tile_pool(name="tp", bufs=8))
        zpool = ctx.enter_context(tc.tile_pool(name="zp", bufs=3))
        opool = ctx.enter_context(tc.tile_pool(name="op", bufs=1))
        pft_pool = ctx.enter_context(tc.tile_pool(name="pft", bufs=1, space="PSUM"))

        # consts dispatched off the sync engine so the x stream starts at once
        w_sb = const_pool.tile([P, NSW * NCHUNK * NF], F32)
        nc.scalar.dma_start(out=w_sb, in_=w)
        bias_sb = const_pool.tile([1, OUT_F], F32)
        nc.scalar.dma_start(out=bias_sb, in_=bias)
        ones = const_pool.tile([1, P], F32)
        nc.gpsimd.memset(ones, 1.0)

        out_sb = opool.tile([BS, OUT_F], F32)
        psum_feat = [
            pft_pool.tile([P, NF], F32, tag=f"feat{s}", name=f"psum_feat{s}")
            for s in range(NSW)
        ]

        for s in range(NSW):
            for g in range(NG):
                # --- load the 8 c-slices of this (s, g): contiguous [128, 1024] ---
                tiles = []
                for c in range(NBINS):
                    t = xpool.tile([P, GW], F32, tag="x", name=f"x_{s}_{g}_{c}")
                    nc.sync.dma_start(out=t, in_=x[(s * NBINS + c) * NG + g])
                    tiles.append(t)
                # --- binary tree c-reduction on DVE + GPSIMD ---
                # lvl0: (01)->DVE (23)->GP (45)->GP (67)->DVE   (67 gates the tail)
                l0 = []
                engs = [nc.vector, nc.gpsimd, nc.gpsimd, nc.vector]
                for j in range(4):
                    u = tpool.tile([P, GW], F32, tag="tree", name=f"t_{s}_{g}_{j}")
                    engs[j].tensor_add(out=u, in0=tiles[2 * j], in1=tiles[2 * j + 1])
                    l0.append(u)
                v0 = tpool.tile([P, GW], F32, tag="tree", name=f"v0_{s}_{g}")
                nc.gpsimd.tensor_add(out=v0, in0=l0[0], in1=l0[1])
                v1 = tpool.tile([P, GW], F32, tag="tree", name=f"v1_{s}_{g}")
                nc.vector.tensor_add(out=v1, in0=l0[2], in1=l0[3])
                z = zpool.tile([P, GW], F32, tag="z", name=f"z_{s}_{g}")
                nc.vector.tensor_add(out=z, in0=v0, in1=v1)

                # --- PE: each 128-col slice of z is a ready lhsT chunk ---
                for j in range(CPG):
                    ch = g * CPG + j
                    nc.tensor.matmul(
                        psum_feat[s],
                        lhsT=z[:, j * P:(j + 1) * P],
                        rhs=w_sb[:, (s * NCHUNK + ch) * NF:(s * NCHUNK + ch + 1) * NF],
                        start=(ch == 0),
                        stop=False,
                    )

        for s in range(NSW):
            # bias via rank-1 matmul: ones[1, b].T @ bias[1, o]
            nc.tensor.matmul(
                psum_feat[s],
                lhsT=ones,
                rhs=bias_sb[:, s * NF:(s + 1) * NF],
                start=False,
                stop=True,
            )
            # LeakyReLU(v) = max(v, slope*v)  (slope < 1)
            tmp = tpool.tile([P, NF], F32, tag="lrelu", name=f"lr_{s}")
            nc.vector.tensor_scalar_mul(tmp, psum_feat[s], SLOPE)
            nc.vector.tensor_max(
                out=out_sb[:, s * NF:(s + 1) * NF], in0=psum_feat[s], in1=tmp
            )

        nc.sync.dma_start(out=out, in_=out_sb)


def _build():
    global _cached
    if _cached is not None:
        return _cached
    nc = bacc.Bacc(
        "TRN2",
        target_bir_lowering=False,
        debug=False,
        enable_asserts=False,
        num_devices=NCORES,
    )
    x_ap = nc.dram_tensor(
        "x", [NSW * NBINS * NG, P, GW], F32, kind="ExternalInput"
    ).ap()
    w_ap = nc.dram_tensor("w", [P, NSW * NCHUNK * NF], F32, kind="ExternalInput").ap()
    b_ap = nc.dram_tensor("bias", [1, OUT_F], F32, kind="ExternalInput").ap()
    out_ap = nc.dram_tensor("out", [BS, OUT_F], F32, kind="ExternalOutput").ap()
    with tile.TileContext(nc, trace_sim=False) as tc:
        _kernel_body(tc, x_ap, w_ap, b_ap, out_ap)
    nc.compile()
    _cached = nc
    return nc


def kernel(x, W, b):
    global last_results
    assert x.shape == (B, 1, NSW * NBINS * NDCT, HW, HW), x.shape
    nc = _build()

    # Host-side folding of the DCT matrices into the conv weights (tiny).
    Ct = _dct2(NDCT)                       # [f, t]
    Cs = _dct2(HW)                         # [p, i]
    Weff = np.einsum(
        "ft,pi,qj,sofpq->sotij", Ct, Cs, Cs, W.astype(np.float64), optimize=True
    ) / float(NBINS)
    Weff_k = Weff.reshape(NSW, NF, K)      # [s, o, k]
    # device layout: w[p, s*NCHUNK*NF + ch*NF + o] = Weff_k[s, o, ch*128 + p]
    w_dev = np.ascontiguousarray(
        Weff_k.reshape(NSW, NF, NCHUNK, P).transpose(3, 0, 2, 1).reshape(P, NSW * NCHUNK * NF)
    ).astype(np.float32)
    bias_dev = np.ascontiguousarray(b.reshape(1, OUT_F)).astype(np.float32)

    x2 = x.reshape(B, NSW, NBINS, NG, CPG, P)  # (b, s, c, g, chin, kin)
    in_maps = []
    for i in range(NCORES):
        xs = x2[i * BS:(i + 1) * BS]
        # -> [s, c, g, kin, chin, b]: one contiguous [128, 1024] block per (s,c,g)
        xt = np.ascontiguousarray(xs.transpose(1, 2, 3, 5, 4, 0)).reshape(
            NSW * NBINS * NG, P, GW
        )
        in_maps.append({"x": xt, "w": w_dev, "bias": bias_dev})
    res = run_bass_kernel_spmd(nc, in_maps, core_ids=list(range(NCORES)))
    last_results = res
    return np.concatenate([r["out"] for r in res.results], axis=0)



# revision 2
# speedup vs baseline: 1.4239x; 1.4239x over previous
"""Trainium2 Bass kernel for nn_DCTFeatureModel.

Math: the reference pipeline (3D DCT-II over [time-in-bin, H, W], mean over
DCT bins, full-receptive-field Conv3d, bias, LeakyReLU) is linear up to the
LeakyReLU, so everything folds into a single small matmul:

    feat[b,s,o] = LeakyReLU( sum_{c,t,i,j} x[b,s,c,t,i,j] * Weff[s,o,t,i,j]
                             + bias[s,o] )
    Weff[s,o,t,i,j] = (1/8) * sum_{f,p,q} Ct[f,t] Cs[p,i] Cs[q,j] W[s,o,f,p,q]

Weff is tiny (2*64*2048 floats) and computed on host. The device kernel is
memory-bound: stream x, reduce over the 8 DCT bins (c), then a small matmul.

v2 device dataflow (per core): x is shipped as bf16 (halves HBM traffic; the
2e-2 rel-err budget dwarfs bf16 quantization). The 8 bins per subwindow are
grouped in 4 pairs; each pair is one contiguous [128 kin, 2(h) x 16(ch) x
128(b)] HBM block. The h=0 half lands via a plain HWDGE DMA into u[s,m]; the
h=1 half lands via a gpsimd SWDGE DMA with accum_op=add (CCE inline add), so
the pair reduction costs zero vector-engine work. The remaining 4-way bin sum
rides the PE's PSUM accumulation: 16 chunk-matmuls per (s,m) tile, 128 total,
plus a rank-1 bias matmul. LeakyReLU on the scalar engine, single out DMA.

All x buffers are dedicated (no tile recycling), so the DMA stream is never
gated by compute - the 16 SDMA engines just drain ~8.4 MB at HBM line rate.

Sharding: pure data-parallel over batch, 1024/8 = 128 rows per core.
"""

from contextlib import ExitStack

import ml_dtypes
import numpy as np

import concourse.bacc as bacc
import concourse.tile as tile
from concourse import mybir
from concourse.bass_utils import run_bass_kernel_spmd

# Problem shapes (hardcoded per contract)
B = 1024
NCORES = 8
BS = B // NCORES          # 128 batch rows per core
NSW = 2                   # subwindows
NBINS = 8                 # DCT bins (mean-reduced)
NPAIR = NBINS // 2        # 4 bin-pairs per subwindow
NDCT = 32                 # time points per bin
HW = 8
NF = 64                   # conv output filters per subwindow
K = NDCT * HW * HW        # 2048 contraction elements per (s, c)
P = 128                   # partitions
NCHUNK = K // P           # 16 k-chunks of 128
OUT_F = NSW * NF          # 128 output features
SLOPE = 0.02

F32 = mybir.dt.float32
BF16 = mybir.dt.bfloat16

# Pair reduction: "dma" = SWDGE accum_op DMA, "vector" = DVE/GPSIMD adds
REDUCE_MODE = "dma"

_cached = None
last_results = None


def _dct2(N):
    n = np.arange(N, dtype=np.float64)
    k = np.arange(N, dtype=np.float64)
    return 2.0 * np.cos(np.pi * (2.0 * n[None, :] + 1.0) * k[:, None] / (2.0 * N))


def _kernel_body(tc, x, w, bias, out):
    """x: [NSW*NPAIR, 128, 2*NCHUNK*BS] bf16, one (s, pair) block per row,
    cols = (h, ch, b); w: [P, NSW*NCHUNK*NF] bf16; bias: [1, OUT_F] bf16;
    out: [BS, OUT_F] f32."""
    nc = tc.nc
    with ExitStack() as ctx:
        const_pool = ctx.enter_context(tc.tile_pool(name="const", bufs=1))
        upool = ctx.enter_context(tc.tile_pool(name="up", bufs=1))
        opool = ctx.enter_context(tc.tile_pool(name="op", bufs=1))
        pft_pool = ctx.enter_context(tc.tile_pool(name="pft", bufs=1, space="PSUM"))

        w_sb = const_pool.tile([P, NSW * NCHUNK * NF], BF16)
        nc.scalar.dma_start(out=w_sb, in_=w)
        bias_sb = const_pool.tile([1, OUT_F], BF16)
        nc.scalar.dma_start(out=bias_sb, in_=bias)
        ones = const_pool.tile([1, BS], BF16)
        nc.vector.memset(ones, 1.0)

        out_sb = opool.tile([BS, OUT_F], F32)
        psum_feat = [
            pft_pool.tile([BS, NF], F32, tag=f"feat{s}", name=f"psum_feat{s}")
            for s in range(NSW)
        ]

        GW = NCHUNK * BS  # 2048 cols per reduced (s, pair) tile

        if REDUCE_MODE == "dma":
            # h=0 half: plain write DMA; h=1 half: SWDGE accum-add DMA.
            utiles = {}
            for s in range(NSW):
                for m in range(NPAIR):
                    u = upool.tile([P, GW], BF16, tag=f"u{s}_{m}", name=f"u_{s}_{m}")
                    utiles[(s, m)] = u
                    eng = nc.sync if s == 0 else nc.scalar
                    eng.dma_start(out=u, in_=x[s * NPAIR + m, :, 0:GW])
            for s in range(NSW):
                for m in range(NPAIR):
                    nc.gpsimd.dma_start(
                        out=utiles[(s, m)],
                        in_=x[s * NPAIR + m, :, GW:2 * GW],
                        accum_op=mybir.AluOpType.add,
                    )
        else:
            xpool = ctx.enter_context(tc.tile_pool(name="xp", bufs=1))
            utiles = {}
            adds = []
            for s in range(NSW):
                for m in range(NPAIR):
                    t = xpool.tile([P, 2 * GW], BF16, tag=f"x{s}_{m}", name=f"x_{s}_{m}")
                    eng = nc.sync if s == 0 else nc.scalar
                    eng.dma_start(out=t, in_=x[s * NPAIR + m])
                    u = upool.tile([P, GW], BF16, tag=f"u{s}_{m}", name=f"u_{s}_{m}")
                    utiles[(s, m)] = u
                    adds.append((u, t))
            # 6 adds on DVE, 2 (mid-stream ones) on GPSIMD
            for j, (u, t) in enumerate(adds):
                eng = nc.gpsimd if j in (1, 5) else nc.vector
                eng.tensor_add(out=u, in0=t[:, 0:GW], in1=t[:, GW:2 * GW])

        for s in range(NSW):
            for m in range(NPAIR):
                u = utiles[(s, m)]
                for ch in range(NCHUNK):
                    nc.tensor.matmul(
                        psum_feat[s],
                        lhsT=u[:, ch * P:(ch + 1) * P],
                        rhs=w_sb[:, (s * NCHUNK + ch) * NF:(s * NCHUNK + ch + 1) * NF],
                        start=(m == 0 and ch == 0),
                        stop=False,
                    )
            nc.tensor.matmul(
                psum_feat[s],
                lhsT=ones,
                rhs=bias_sb[:, s * NF:(s + 1) * NF],
                start=False,
                stop=True,
            )
            nc.scalar.activation(
                out=out_sb[:, s * NF:(s + 1) * NF],
                in_=psum_feat[s],
                func=mybir.ActivationFunctionType.Lrelu,
                alpha=SLOPE,
            )

        nc.sync.dma_start(out=out, in_=out_sb)


def _build():
    global _cached
    if _cached is not None:
        return _cached
    nc = bacc.Bacc(
        "TRN2",
        target_bir_lowering=False,
        debug=False,
        enable_asserts=False,
        num_devices=NCORES,
    )
    x_ap = nc.dram_tensor(
        "x", [NSW * NPAIR, P, 2 * NCHUNK * BS], BF16, kind="ExternalInput"
    ).ap()
    w_ap = nc.dram_tensor("w", [P, NSW * NCHUNK * NF], BF16, kind="ExternalInput").ap()
    b_ap = nc.dram_tensor("bias", [1, OUT_F], BF16, kind="ExternalInput").ap()
    out_ap = nc.dram_tensor("out", [BS, OUT_F], F32, kind="ExternalOutput").ap()
    with tile.TileContext(nc, trace_sim=False) as tc:
        _kernel_body(tc, x_ap, w_ap, b_ap, out_ap)
    nc.compile()
    _cached = nc
    return nc


def kernel(x, W, b):
    global last_results
    assert x.shape == (B, 1, NSW * NBINS * NDCT, HW, HW), x.shape
    nc = _build()

    # Host-side folding of the DCT matrices into the conv weights (tiny).
    Ct = _dct2(NDCT)                       # [f, t]
    Cs = _dct2(HW)                         # [p, i]
    Weff = np.einsum(
        "ft,pi,qj,sofpq->sotij", Ct, Cs, Cs, W.astype(np.float64), optimize=True
    ) / float(NBINS)
    Weff_k = Weff.reshape(NSW, NF, K)      # [s, o, k]
    # device layout: w[p, s*NCHUNK*NF + ch*NF + o] = Weff_k[s, o, ch*128 + p]
    w_dev = np.ascontiguousarray(
        Weff_k.reshape(NSW, NF, NCHUNK, P).transpose(3, 0, 2, 1).reshape(P, NSW * NCHUNK * NF)
    ).astype(ml_dtypes.bfloat16)
    bias_dev = np.ascontiguousarray(b.reshape(1, OUT_F)).astype(ml_dtypes.bfloat16)

    # (b, s, m, h, ch, kin) with bin c = 2m + h
    x_bf = x.reshape(B, NSW, NPAIR, 2, NCHUNK, P).astype(ml_dtypes.bfloat16)
    in_maps = []
    for i in range(NCORES):
        xs = x_bf[i * BS:(i + 1) * BS]
        # -> [s, m, kin, h, ch, b]: per (s, m) two contiguous 512 KiB halves
        xt = np.ascontiguousarray(xs.transpose(1, 2, 5, 3, 4, 0)).reshape(
            NSW * NPAIR, P, 2 * NCHUNK * BS
        )
        in_maps.append({"x": xt, "w": w_dev, "bias": bias_dev})
    res = run_bass_kernel_spmd(nc, in_maps, core_ids=list(range(NCORES)))
    last_results = res
    return np.concatenate([r["out"] for r in res.results], axis=0)


# revision 4
# speedup vs baseline: 1.7265x; 1.2126x over previous
"""Trainium2 Bass kernel for nn_DCTFeatureModel.

Math: the reference pipeline (3D DCT-II over [time-in-bin, H, W], mean over
DCT bins, full-receptive-field Conv3d, bias, LeakyReLU) is linear up to the
LeakyReLU, so everything folds into a single small matmul:

    feat[b,s,o] = LeakyReLU( sum_{c,t,i,j} x[b,s,c,t,i,j] * Weff[s,o,t,i,j]
                             + bias[s,o] )
    Weff[s,o,t,i,j] = (1/8) * sum_{f,p,q} Ct[f,t] Cs[p,i] Cs[q,j] W[s,o,f,p,q]

Weff is tiny (2*64*2048 floats) and computed on host. The device kernel is
memory-bound: stream x, reduce over the 8 DCT bins (c), then a small matmul.

v2 device dataflow (per core): x is shipped as bf16 (halves HBM traffic; the
2e-2 rel-err budget dwarfs bf16 quantization). The 8 bins per subwindow are
grouped in 4 pairs; each pair is one contiguous [128 kin, 2(h) x 16(ch) x
128(b)] HBM block. The h=0 half lands via a plain HWDGE DMA into u[s,m]; the
h=1 half lands via a gpsimd SWDGE DMA with accum_op=add (CCE inline add), so
the pair reduction costs zero vector-engine work. The remaining 4-way bin sum
rides the PE's PSUM accumulation: 16 chunk-matmuls per (s,m) tile, 128 total,
plus a rank-1 bias matmul. LeakyReLU on the scalar engine, single out DMA.

All x buffers are dedicated (no tile recycling), so the DMA stream is never
gated by compute - the 16 SDMA engines just drain ~8.4 MB at HBM line rate.

Sharding: pure data-parallel over batch, 1024/8 = 128 rows per core.
"""

from contextlib import ExitStack

import ml_dtypes
import numpy as np

import concourse.bacc as bacc
import concourse.tile as tile
from concourse import mybir
from concourse.bass_utils import run_bass_kernel_spmd

# Problem shapes (hardcoded per contract)
B = 1024
NCORES = 8
BS = B // NCORES          # 128 batch rows per core
NSW = 2                   # subwindows
NBINS = 8                 # DCT bins (mean-reduced)
NPAIR = NBINS // 2        # 4 bin-pairs per subwindow
NDCT = 32                 # time points per bin
HW = 8
NF = 64                   # conv output filters per subwindow
K = NDCT * HW * HW        # 2048 contraction elements per (s, c)
P = 128                   # partitions
NCHUNK = K // P           # 16 k-chunks of 128
OUT_F = NSW * NF          # 128 output features
SLOPE = 0.02

F32 = mybir.dt.float32
BF16 = mybir.dt.bfloat16

# Pair reduction: "dma" = SWDGE accum_op DMA, "vector" = DVE/GPSIMD adds.
# "dma" measured slower (Q7 SWDGE issue serializes ~1us/DMA + completion
# waits -> SDMA engines starve) and less accurate (CCE bf16 add).
REDUCE_MODE = "vector"

_cached = None
last_results = None


def _dct2(N):
    n = np.arange(N, dtype=np.float64)
    k = np.arange(N, dtype=np.float64)
    return 2.0 * np.cos(np.pi * (2.0 * n[None, :] + 1.0) * k[:, None] / (2.0 * N))


def _kernel_body(tc, x, w, bias, out):
    """x: [NSW*NPAIR, 128, 2*NCHUNK*BS] bf16, one (s, pair) block per row,
    cols = (h, ch, b); w: [P, NSW*NCHUNK*NF] bf16; bias: [1, OUT_F] bf16;
    out: [BS, OUT_F] f32."""
    nc = tc.nc
    with ExitStack() as ctx:
        const_pool = ctx.enter_context(tc.tile_pool(name="const", bufs=1))
        upool = ctx.enter_context(tc.tile_pool(name="up", bufs=1))
        opool = ctx.enter_context(tc.tile_pool(name="op", bufs=1))
        pft_pool = ctx.enter_context(tc.tile_pool(name="pft", bufs=1, space="PSUM"))

        w_sb = const_pool.tile([P, NSW * NCHUNK * NF], BF16)
        nc.scalar.dma_start(out=w_sb, in_=w)
        bias_sb = const_pool.tile([1, OUT_F], BF16)
        nc.scalar.dma_start(out=bias_sb, in_=bias)
        ones = const_pool.tile([1, BS], BF16)
        nc.vector.memset(ones, 1.0)

        out_sb = opool.tile([BS, OUT_F], F32)
        psum_feat = [
            pft_pool.tile([BS, NF], F32, tag=f"feat{s}", name=f"psum_feat{s}")
            for s in range(NSW)
        ]

        GW = NCHUNK * BS  # 2048 cols per reduced (s, pair) tile

        if REDUCE_MODE == "dma":
            # h=0 half: plain write DMA; h=1 half: SWDGE accum-add DMA.
            utiles = {}
            for s in range(NSW):
                for m in range(NPAIR):
                    u = upool.tile([P, GW], BF16, tag=f"u{s}_{m}", name=f"u_{s}_{m}")
                    utiles[(s, m)] = u
                    eng = nc.sync if s == 0 else nc.scalar
                    eng.dma_start(out=u, in_=x[s * NPAIR + m, :, 0:GW])
            for s in range(NSW):
                for m in range(NPAIR):
                    nc.gpsimd.dma_start(
                        out=utiles[(s, m)],
                        in_=x[s * NPAIR + m, :, GW:2 * GW],
                        accum_op=mybir.AluOpType.add,
                    )
        else:
            xpool = ctx.enter_context(tc.tile_pool(name="xp", bufs=1))
            utiles = {}
            adds = []
            for s in range(NSW):
                for m in range(NPAIR):
                    t = xpool.tile([P, 2 * GW], BF16, tag=f"x{s}_{m}", name=f"x_{s}_{m}")
                    eng = nc.sync if s == 0 else nc.scalar
                    eng.dma_start(out=t, in_=x[s * NPAIR + m])
                    u = upool.tile([P, GW], BF16, tag=f"u{s}_{m}", name=f"u_{s}_{m}")
                    utiles[(s, m)] = u
                    adds.append((u, t))
            # 6 adds on DVE, 2 (mid-stream ones) on GPSIMD
            for j, (u, t) in enumerate(adds):
                eng = nc.gpsimd if j in (1, 5) else nc.vector
                eng.tensor_add(out=u, in0=t[:, 0:GW], in1=t[:, GW:2 * GW])

        for s in range(NSW):
            for m in range(NPAIR):
                u = utiles[(s, m)]
                for ch in range(NCHUNK):
                    nc.tensor.matmul(
                        psum_feat[s],
                        lhsT=u[:, ch * P:(ch + 1) * P],
                        rhs=w_sb[:, (s * NCHUNK + ch) * NF:(s * NCHUNK + ch + 1) * NF],
                        start=(m == 0 and ch == 0),
                        stop=False,
                    )
            nc.tensor.matmul(
                psum_feat[s],
                lhsT=ones,
                rhs=bias_sb[:, s * NF:(s + 1) * NF],
                start=False,
                stop=True,
            )
            # LeakyReLU(v) = max(v, slope*v), exact on DVE (scalar-engine
            # Lrelu is table-based and costs ~3x the error)
            tmp = upool.tile([BS, NF], F32, tag=f"lr{s}", name=f"lr_{s}")
            nc.vector.tensor_scalar_mul(tmp, psum_feat[s], SLOPE)
            nc.vector.tensor_max(
                out=out_sb[:, s * NF:(s + 1) * NF], in0=psum_feat[s], in1=tmp
            )

        nc.sync.dma_start(out=out, in_=out_sb)


def _build():
    global _cached
    if _cached is not None:
        return _cached
    nc = bacc.Bacc(
        "TRN2",
        target_bir_lowering=False,
        debug=False,
        enable_asserts=False,
        num_devices=NCORES,
    )
    x_ap = nc.dram_tensor(
        "x", [NSW * NPAIR, P, 2 * NCHUNK * BS], BF16, kind="ExternalInput"
    ).ap()
    w_ap = nc.dram_tensor("w", [P, NSW * NCHUNK * NF], BF16, kind="ExternalInput").ap()
    b_ap = nc.dram_tensor("bias", [1, OUT_F], BF16, kind="ExternalInput").ap()
    out_ap = nc.dram_tensor("out", [BS, OUT_F], F32, kind="ExternalOutput").ap()
    with tile.TileContext(nc, trace_sim=False) as tc:
        _kernel_body(tc, x_ap, w_ap, b_ap, out_ap)
    nc.compile()
    _cached = nc
    return nc


def kernel(x, W, b):
    global last_results
    assert x.shape == (B, 1, NSW * NBINS * NDCT, HW, HW), x.shape
    nc = _build()

    # Host-side folding of the DCT matrices into the conv weights (tiny).
    Ct = _dct2(NDCT)                       # [f, t]
    Cs = _dct2(HW)                         # [p, i]
    Weff = np.einsum(
        "ft,pi,qj,sofpq->sotij", Ct, Cs, Cs, W.astype(np.float64), optimize=True
    ) / float(NBINS)
    Weff_k = Weff.reshape(NSW, NF, K)      # [s, o, k]
    # device layout: w[p, s*NCHUNK*NF + ch*NF + o] = Weff_k[s, o, ch*128 + p]
    w_dev = np.ascontiguousarray(
        Weff_k.reshape(NSW, NF, NCHUNK, P).transpose(3, 0, 2, 1).reshape(P, NSW * NCHUNK * NF)
    ).astype(ml_dtypes.bfloat16)
    bias_dev = np.ascontiguousarray(b.reshape(1, OUT_F)).astype(ml_dtypes.bfloat16)

    # (b, s, m, h, ch, kin) with bin c = 2m + h
    x_bf = x.reshape(B, NSW, NPAIR, 2, NCHUNK, P).astype(ml_dtypes.bfloat16)
    in_maps = []
    for i in range(NCORES):
        xs = x_bf[i * BS:(i + 1) * BS]
        # -> [s, m, kin, h, ch, b]: per (s, m) two contiguous 512 KiB halves
        xt = np.ascontiguousarray(xs.transpose(1, 2, 5, 3, 4, 0)).reshape(
            NSW * NPAIR, P, 2 * NCHUNK * BS
        )
        in_maps.append({"x": xt, "w": w_dev, "bias": bias_dev})
    res = run_bass_kernel_spmd(nc, in_maps, core_ids=list(range(NCORES)))
    last_results = res
    return np.concatenate([r["out"] for r in res.results], axis=0)
